# revision 13
# baseline (speedup 1.0000x reference)
"""Trainium2 Bass kernel for GQA attention (B=2, S=2048, D=2048, H=16, KVH=4).

Sharding: 8 cores = (batch b in {0,1}) x (kv-group g in {0..3}).
Core c = b*4 + g computes q-heads 4g..4g+3 against kv-head g for batch b,
producing a partial output projection res_partial.T = [e=2048, s=2048];
host sums the 4 partials per batch.

Device layout notes (per core):
  - All projection/attention matmuls in fp32r (TF32-like, full PE rate at N>=512).
  - qT/kT layout: [head_dim on partitions, seq on free] -> QK^T and PV need no
    transposes; softmax denominator via ones-column matmul on the PE.
  - scores computed transposed: S.T[k_pos, q_pos]; exp on ACT without
    max-subtraction (|score| <= sqrt(128), safe for gain==1);
    causal mask via gpsimd affine_select on diagonal blocks.
  - RMS-norm partition sums + 1/rms broadcast also via tiny PE matmuls.
"""

import sys

sys.path.insert(0, "/opt/trn_rl_repo")

from contextlib import ExitStack

import numpy as np

import concourse.bass as bass
import concourse.tile as tile
from concourse import bacc, mybir
from concourse import bass_utils

B, S, D = 2, 2048, 2048
H, KVH = 16, 4
HD = 128               # head dim
GQ = 4                 # q heads per core
SL = GQ * HD           # 512: q-head slice width per core
NCORES = 8
SC = S // 512          # 4 s-chunks of 512
KC = D // 128          # 16 d-chunks of 128
ROPE_BASE = 10000.0
EPS = 1.1920929e-07
F32 = mybir.dt.float32
F32R = mybir.dt.float32r

_COMPILED_NC = None


def _build_body(tc):
    nc = tc.nc
    ctx = ExitStack()
    ctx.enter_context(nc.allow_low_precision(reason="fp32r matmul operand tiles"))

    xT = nc.dram_tensor("xT", [D, S], F32, kind="ExternalInput").ap()
    wqT = nc.dram_tensor("wqT", [D, SL], F32, kind="ExternalInput").ap()
    wkT = nc.dram_tensor("wkT", [D, HD], F32, kind="ExternalInput").ap()
    wvT = nc.dram_tensor("wvT", [D, HD], F32, kind="ExternalInput").ap()
    woA = nc.dram_tensor("woA", [SL, D], F32, kind="ExternalInput").ap()
    csd = nc.dram_tensor("csd", [128, S], F32, kind="ExternalInput").ap()
    snd = nc.dram_tensor("snd", [128, S], F32, kind="ExternalInput").ap()
    bqkd = nc.dram_tensor("bqkd", [128, GQ + 1], F32, kind="ExternalInput").ap()
    constd = nc.dram_tensor("constd", [257], F32, kind="ExternalInput").ap()
    identd = nc.dram_tensor("identd", [128, 128], F32, kind="ExternalInput").ap()
    gaind = nc.dram_tensor("gaind", [GQ * 128], F32, kind="ExternalInput").ap()
    resT = nc.dram_tensor("resT", [D, S], F32, kind="ExternalOutput").ap()

    persist = ctx.enter_context(tc.tile_pool(name="persist", bufs=1))
    xpool = ctx.enter_context(tc.tile_pool(name="xpool", bufs=4))
    wqpool = ctx.enter_context(tc.tile_pool(name="wqpool", bufs=6))
    wopool = ctx.enter_context(tc.tile_pool(name="wopool", bufs=8))
    bpool = ctx.enter_context(tc.tile_pool(name="bpool", bufs=2))
    rowp = ctx.enter_context(tc.tile_pool(name="rowp", bufs=2))
    expp = ctx.enter_context(tc.tile_pool(name="expp", bufs=3))
    otp = ctx.enter_context(tc.tile_pool(name="otp", bufs=2))
    resp = ctx.enter_context(tc.tile_pool(name="resp", bufs=2))
    vtp = ctx.enter_context(tc.tile_pool(name="vtp", bufs=2))
    psA = ctx.enter_context(tc.tile_pool(name="psA", bufs=3, space="PSUM"))
    psB = ctx.enter_context(tc.tile_pool(name="psB", bufs=1, space="PSUM"))
    psS = ctx.enter_context(tc.tile_pool(name="psS", bufs=2, space="PSUM"))
    psO = ctx.enter_context(tc.tile_pool(name="psO", bufs=1, space="PSUM"))
    psD = ctx.enter_context(tc.tile_pool(name="psD", bufs=1, space="PSUM"))

    # ---- constants / persistent tiles (DMA'd: memset cannot write fp32r) ----
    ones_col = persist.tile([128, 1], F32R, name="ones_col")
    nc.sync.dma_start(ones_col, bass.AP(tensor=constd.tensor, offset=0,
                                        ap=[[1, 128], [1, 1]]).bitcast(F32R))
    ones_row = persist.tile([1, 128], F32R, name="ones_row")
    nc.sync.dma_start(ones_row, bass.AP(tensor=constd.tensor, offset=0,
                                        ap=[[0, 1], [1, 128]]).bitcast(F32R))
    kscale_row = persist.tile([1, 128], F32R, name="kscale_row")
    nc.sync.dma_start(kscale_row, bass.AP(tensor=constd.tensor, offset=128,
                                          ap=[[0, 1], [1, 128]]).bitcast(F32R))
    eps_col = persist.tile([1, 1], F32, name="eps_col")
    nc.sync.dma_start(eps_col, bass.AP(tensor=constd.tensor, offset=256,
                                       ap=[[0, 1], [1, 1]]))

    gcol = persist.tile([1, GQ, 128], F32R, name="gcol")
    nc.sync.dma_start(
        gcol,
        bass.AP(tensor=gaind.tensor, offset=0,
                ap=[[0, 1], [128, GQ], [1, 128]]).bitcast(F32R),
    )
    bqcols = persist.tile([128, GQ + 1], F32, name="bqcols")
    nc.sync.dma_start(bqcols, bqkd)

    cs_sb = persist.tile([128, S], F32, name="cs_sb")
    nc.sync.dma_start(cs_sb, csd)
    sn_sb = persist.tile([128, S], F32, name="sn_sb")
    nc.sync.dma_start(sn_sb, snd)

    wk_sb = persist.tile([128, KC, HD], F32R, name="wk_sb")
    nc.sync.dma_start(
        wk_sb, wkT.rearrange("(kc p) h -> p kc h", p=128).bitcast(F32R)
    )
    wv_sb = persist.tile([128, KC, HD], F32R, name="wv_sb")
    nc.sync.dma_start(
        wv_sb, wvT.rearrange("(kc p) h -> p kc h", p=128).bitcast(F32R)
    )

    ident = persist.tile([128, 128], F32R, name="ident")
    nc.sync.dma_start(ident, identd.bitcast(F32R))

    qfin = [
        persist.tile([128, S], F32R, name=f"qfin{h}", tag=f"qfin{h}") for h in range(GQ)
    ]
    kfin = persist.tile([128, S], F32R, name="kfin")
    v_sb = [
        persist.tile([128, HD], F32R, name=f"vsb{i}", tag=f"vsb{i}") for i in range(KC)
    ]

    # ================= Stage A+B: projections, rms-norm, rope =================
    def stage_b(et, sc, psum_p):
        """et in 0..3 -> q head et;  et == 4 -> k."""
        is_q = et < GQ
        bias_col = bqcols[:, et : et + 1] if is_q else bqcols[:, GQ : GQ + 1]
        q_raw = bpool.tile([128, 512], F32, tag="qraw", bufs=3)
        nc.vector.tensor_scalar_add(q_raw, psum_p, bias_col)
        # sum of squares along head dim (partitions) via ones-matmul
        sq = bpool.tile([128, 512], F32R, tag="sq", bufs=3)
        nc.scalar.square(sq, q_raw)
        ss = psS.tile([1, 512], F32, tag="pS")
        nc.tensor.matmul(ss, ones_col, sq, start=True, stop=True)
        srow = rowp.tile([1, 512], F32R, tag="srow")
        nc.scalar.activation(srow, ss, mybir.ActivationFunctionType.Sqrt,
                             bias=eps_col, scale=1.0 / HD)
        rrow = srow
        nc.vector.reciprocal(rrow, srow)
        # broadcast 1/rms (x gain for q, x 1/sqrt(HD) for k) to 128 partitions
        lcol = gcol[:, et, :] if is_q else kscale_row
        scale_ps = psB.tile([128, 512], F32, tag="pB")
        nc.tensor.matmul(scale_ps, lcol, rrow, start=True, stop=True)
        # rope: swap halves via sbuf->sbuf DMA (sn rows 64..127 hold -sin)
        sw = bpool.tile([128, 512], F32, tag="sw", bufs=2)
        nc.sync.dma_start(sw[0:64, :], q_raw[64:128, :])
        nc.sync.dma_start(sw[64:128, :], q_raw[0:64, :])
        t1 = bpool.tile([128, 512], F32, tag="t1", bufs=2)
        nc.vector.tensor_mul(t1, q_raw, cs_sb[:, sc * 512 : (sc + 1) * 512])
        nc.vector.tensor_mul(sw, sw, sn_sb[:, sc * 512 : (sc + 1) * 512])
        nc.vector.tensor_add(t1, t1, sw)
        dst = qfin[et] if is_q else kfin
        nc.vector.tensor_mul(dst[:, sc * 512 : (sc + 1) * 512], t1, scale_ps)

    for sc in range(SC):
        xq = []
        for kq in range(4):
            xt = xpool.tile([128, 4, 512], F32R, name=f"xq{sc}_{kq}", tag="xq")
            nc.sync.dma_start(
                xt,
                bass.AP(
                    tensor=xT.tensor,
                    offset=kq * 512 * S + sc * 512,
                    ap=[[S, 128], [128 * S, 4], [1, 512]],
                ).bitcast(F32R),
            )
            xq.append(xt)

        # group 0: q heads 0..2 | group 1: q head 3, k, vT
        for grp in ((0, 1, 2), (3, GQ, GQ + 1)):
            psums = {}
            for et in grp:
                psums[et] = psA.tile([128, 512], F32, tag="pA", name=f"psA{sc}_{grp[0]}_{et}")
            wq_tiles = {}
            for kq in range(4):
                for kc4 in range(4):
                    kc = kq * 4 + kc4
                    if grp[0] == 0:
                        wt = wqpool.tile([128, 512], F32R, name=f"wq{sc}_{kc}",
                                         tag="wq", bufs=6)
                        nc.sync.dma_start(
                            wt,
                            bass.AP(tensor=wqT.tensor, offset=kc * 128 * SL,
                                    ap=[[SL, 128], [1, 512]]).bitcast(F32R),
                        )
                    else:
                        wt = wqpool.tile([128, 128], F32R, name=f"wq3{sc}_{kc}",
                                         tag="wq3", bufs=6)
                        nc.sync.dma_start(
                            wt,
                            bass.AP(tensor=wqT.tensor, offset=kc * 128 * SL + 3 * 128,
                                    ap=[[SL, 128], [1, 128]]).bitcast(F32R),
                        )
                    wq_tiles[kc] = wt
                    start = kc == 0
                    stop = kc == KC - 1
                    for et in grp:
                        if et < GQ:
                            lhsT = (wq_tiles[kc][:, et * 128 : (et + 1) * 128]
                                    if grp[0] == 0 else wq_tiles[kc])
                            nc.tensor.matmul(psums[et], lhsT, xq[kq][:, kc4, :],
                                             start=start, stop=stop)
                        elif et == GQ:  # k
                            nc.tensor.matmul(psums[et], wk_sb[:, kc, :],
                                             xq[kq][:, kc4, :], start=start, stop=stop)
                        else:  # vT
                            nc.tensor.matmul(psums[et], wv_sb[:, kc, :],
                                             xq[kq][:, kc4, :], start=start, stop=stop)
            for et in grp:
                if et <= GQ:
                    stage_b(et, sc, psums[et])
                else:
                    # vT chunk -> sbuf, then PE-transpose to v[s_tile, dh]
                    vt = vtp.tile([128, 512], F32R, tag="vt", bufs=2)
                    nc.vector.tensor_copy(vt, psums[et])
                    for j in range(4):
                        stile = sc * 4 + j
                        pst = psS.tile([128, 128], F32R, tag="pS", name=f"pst{sc}_{j}")
                        nc.tensor.transpose(pst, vt[:, j * 128 : (j + 1) * 128], ident)
                        nc.vector.tensor_copy(v_sb[stile], pst)

    # ================= Stage C: attention | Stage D: output proj =============
    for qc in range(SC):
        otp_tiles = {}
        for h in range(GQ):
            nblk = 4 * (qc + 1)
            psum_o = psO.tile([128, 512], F32, tag="pO")
            psum_d = psD.tile([1, 512], F32, tag="pD")
            pend = None  # software-pipeline PV/denom one block behind
            for kt in range(nblk):
                ps_s = psS.tile([128, 512], F32, tag="pS")
                nc.tensor.matmul(
                    ps_s,
                    kfin[:, kt * 128 : (kt + 1) * 128],
                    qfin[h][:, qc * 512 : (qc + 1) * 512],
                    start=True, stop=True,
                )
                exp_s = expp.tile([128, 512], F32R, tag="exp")
                nc.scalar.activation(exp_s, ps_s, mybir.ActivationFunctionType.Exp)
                if kt >= qc * 4:  # diagonal block: causal mask (keep q >= k)
                    nc.gpsimd.affine_select(
                        out=exp_s, in_=exp_s,
                        pattern=[[1, 512]],
                        compare_op=mybir.AluOpType.is_ge,
                        fill=0.0,
                        base=qc * 512 - kt * 128,
                        channel_multiplier=-1,
                    )
                if pend is not None:
                    pkt, pexp = pend
                    nc.tensor.matmul(psum_o, v_sb[pkt], pexp,
                                     start=(pkt == 0), stop=False)
                    nc.tensor.matmul(psum_d, ones_col, pexp,
                                     start=(pkt == 0), stop=False)
                pend = (kt, exp_s)
            pkt, pexp = pend
            nc.tensor.matmul(psum_o, v_sb[pkt], pexp, start=(pkt == 0), stop=True)
            nc.tensor.matmul(psum_d, ones_col, pexp, start=(pkt == 0), stop=True)
            # normalize: O.T = O'.T * (1/denom) broadcast across partitions
            rrow2 = rowp.tile([1, 512], F32R, tag="rrow2")
            nc.vector.reciprocal(rrow2, psum_d)
            ps_r = psB.tile([128, 512], F32, tag="pB")
            nc.tensor.matmul(ps_r, ones_row, rrow2, start=True, stop=True)
            rb = bpool.tile([128, 512], F32, tag="rb", bufs=2)
            nc.scalar.copy(rb, ps_r)
            ot = otp.tile([128, 512], F32R, tag=f"ot{h}", bufs=2)
            nc.vector.tensor_mul(ot, psum_o, rb)
            otp_tiles[h] = ot

        # Stage D for this qc: res.T[e, qc] = sum_h woA_h.T @ O_h.T
        for etg in range(8):
            wo_t = []
            for h in range(GQ):
                wt = wopool.tile([128, 256], F32R, name=f"wo{qc}_{etg}_{h}",
                                 tag="wo", bufs=8)
                nc.sync.dma_start(
                    wt,
                    bass.AP(tensor=woA.tensor, offset=h * 128 * D + etg * 256,
                            ap=[[D, 128], [1, 256]]).bitcast(F32R),
                )
                wo_t.append(wt)
            for e2 in range(2):
                et = etg * 2 + e2
                ps_res = psA.tile([128, 512], F32, tag="pA")
                for h in range(GQ):
                    nc.tensor.matmul(
                        ps_res, wo_t[h][:, e2 * 128 : (e2 + 1) * 128],
                        otp_tiles[h],
                        start=(h == 0), stop=(h == GQ - 1),
                    )
                r = resp.tile([128, 512], F32, tag="res")
                nc.vector.tensor_copy(r, ps_res)
                nc.sync.dma_start(
                    resT[et * 128 : (et + 1) * 128, qc * 512 : (qc + 1) * 512], r
                )

    ctx.close()


def _build():
    global _COMPILED_NC
    if _COMPILED_NC is not None:
        return _COMPILED_NC
    nc = bacc.Bacc("TRN2", target_bir_lowering=False, debug=False,
                   num_devices=NCORES)
    with tile.TileContext(nc) as tc:
        _build_body(tc)
    nc.compile()
    _COMPILED_NC = nc
    return nc


def _rope_tables():
    inv_freq = 1.0 / (ROPE_BASE ** (np.arange(0, HD, 2, dtype=np.float64) / HD))
    t = np.arange(S, dtype=np.float64)
    freqs = np.outer(t, inv_freq)          # [S, 64]
    cos = np.cos(freqs).T.astype(np.float32)   # [64, S]
    sin = np.sin(freqs).T.astype(np.float32)
    cs = np.concatenate([cos, cos], axis=0)    # [128, S]
    sn = np.concatenate([sin, -sin], axis=0)   # [128, S] (bottom half negated)
    return np.ascontiguousarray(cs), np.ascontiguousarray(sn)


def kernel(x, Wq, bq, Wk, bk, Wv, bv, Wo, bo, q_gain):
    x = np.asarray(x, np.float32)
    Wq = np.asarray(Wq, np.float32)
    bq = np.asarray(bq, np.float32)
    Wk = np.asarray(Wk, np.float32)
    bk = np.asarray(bk, np.float32)
    Wv = np.asarray(Wv, np.float32)
    bv = np.asarray(bv, np.float32)
    Wo = np.asarray(Wo, np.float32)
    bo = np.asarray(bo, np.float32)
    q_gain = np.asarray(q_gain, np.float32)

    cs, sn = _rope_tables()
    const_arr = np.concatenate([
        np.ones(128, np.float32),
        np.full(128, float(HD) ** -0.5, np.float32),
        np.array([EPS], np.float32),
    ])
    ident_arr = np.eye(128, dtype=np.float32)
    in_maps = []
    for c in range(NCORES):
        b, g = divmod(c, KVH)
        sl = slice(g * SL, (g + 1) * SL)
        hs = slice(g * HD, (g + 1) * HD)
        in_maps.append({
            "xT": np.ascontiguousarray(x[b].T),
            "wqT": np.ascontiguousarray(Wq[sl, :].T),
            "wkT": np.ascontiguousarray(Wk[hs, :].T),
            "wvT": np.ascontiguousarray(Wv[hs, :].T),
            "woA": np.ascontiguousarray(Wo[:, sl].T),
            "csd": cs, "snd": sn,
            "bqkd": np.ascontiguousarray(np.concatenate(
                [bq[sl].reshape(GQ, HD).T, bk[hs].reshape(1, HD).T], axis=1)),
            "gaind": np.ascontiguousarray(
                np.repeat(q_gain[g * GQ : (g + 1) * GQ], 128)),
            "constd": const_arr, "identd": ident_arr,
        })

    global _LAST_IN_MAPS
    _LAST_IN_MAPS = in_maps
    nc = _build()
    res = bass_utils.run_bass_kernel_spmd(nc, in_maps, core_ids=list(range(NCORES)))

    # v-bias and o-bias folded on host: attention rows sum to 1, so +bv
    # passes through to O exactly; res += bv_rep @ Wo.T + bo.
    bv_rep = np.repeat(bv.reshape(KVH, HD), H // KVH, axis=0).reshape(-1)
    host_const = (Wo @ bv_rep + bo).astype(np.float32)

    out = np.zeros((B, S, D), np.float32)
    for c in range(NCORES):
        b = c // KVH
        out[b] += res.results[c]["resT"].T
    out += host_const[None, None, :]
    return out
